# revision 26
# baseline (speedup 1.0000x reference)
"""Trainium2 Bass kernel for a ViT-style transformer block (B=64,N=197,C=768,H=12,P=20).

Strategy: data-parallel over batch across 8 NeuronCores (8 images/core).
Inside each core: feature-major activations, f32r matmuls for the big GEMMs,
bf16 attention, LN via ones-matmul column sums + gpsimd partition broadcast,
attention with kv-on-partitions scores (S_T = K^T q) so no on-chip transposes
are needed, softmax denominator from an appended ones-column on token-major V.
"""
import numpy as np
import concourse.bass as bass
import concourse.mybir as mybir
import concourse.tile as tile
from concourse import bacc, bass_utils
from contextlib import ExitStack

F32 = mybir.dt.float32
F32R = mybir.dt.float32r
BF16 = mybir.dt.bfloat16
AF = mybir.ActivationFunctionType

B, N, C, H, Dh, P, Dff = 64, 197, 768, 12, 64, 20, 3072
NCORES = 8
BL = B // NCORES          # images per core
KV = P + N                # 217
CT = C // 128             # 6 c-tiles
QP = N + 1                # 198, even padded q width
EPS = 1e-6


def build_nc(bl=BL):
    T = bl * N
    TP = T + 8                 # padded q columns
    nch = max(bl // 2, 1)      # chunks of 2 images
    chunk = T // nch           # 394 (even, >=256)

    nc = bacc.Bacc(trn_type="TRN2", target_bir_lowering=False)

    x_fm = nc.dram_tensor("x_fm", [C, T], F32R, kind="ExternalInput")
    kp = nc.dram_tensor("kp", [CT, 128, bl, P], BF16, kind="ExternalInput")
    vp = nc.dram_tensor("vp", [bl, P, H, Dh], BF16, kind="ExternalInput")
    w_qk = nc.dram_tensor("w_qk", [C, 2 * C], F32R, kind="ExternalInput")
    b_qk = nc.dram_tensor("b_qk", [128, 12], F32, kind="ExternalInput")
    w_v = nc.dram_tensor("w_v", [C, C], F32R, kind="ExternalInput")
    b_v = nc.dram_tensor("b_v", [128, 6], F32, kind="ExternalInput")
    w_pr = nc.dram_tensor("w_pr", [C, C], BF16, kind="ExternalInput")
    b_pr = nc.dram_tensor("b_pr", [128, 6], F32, kind="ExternalInput")
    w_f1 = nc.dram_tensor("w_f1", [C, Dff], F32R, kind="ExternalInput")
    b_f1 = nc.dram_tensor("b_f1", [128, 24], F32, kind="ExternalInput")
    w_f2 = nc.dram_tensor("w_f2", [Dff, C], BF16, kind="ExternalInput")
    b_f2 = nc.dram_tensor("b_f2", [128, 6], F32, kind="ExternalInput")
    out_fm = nc.dram_tensor("out_fm", [C, T], F32, kind="ExternalOutput")

    with tile.TileContext(nc) as tc, ExitStack() as top:
        consts = top.enter_context(tc.tile_pool(name="consts", bufs=1))
        ones_f = consts.tile([128, 8], F32)
        nc.vector.memset(ones_f[:], 1.0)
        zeros_f = consts.tile([128, 8], F32)
        nc.vector.memset(zeros_f[:], 0.0)
        ones_r = consts.tile([128, 1], F32R)
        nc.vector.tensor_copy(ones_r[:], ones_f[:, 0:1])
        eps_sb = consts.tile([1, 1], F32)
        nc.vector.memset(eps_sb[:], EPS)
        bqk_sb = consts.tile([128, 12], F32)
        nc.sync.dma_start(bqk_sb[:], b_qk[:])
        bv_sb = consts.tile([128, 6], F32)
        nc.sync.dma_start(bv_sb[:], b_v[:])
        bpr_sb = consts.tile([128, 6], F32)
        nc.sync.dma_start(bpr_sb[:], b_pr[:])
        bf1_sb = consts.tile([128, 24], F32)
        nc.sync.dma_start(bf1_sb[:], b_f1[:])
        bf2_sb = consts.tile([128, 6], F32)
        nc.sync.dma_start(bf2_sb[:], b_f2[:])

        # single always-open PSUM pool (1 tag, 8 banks) and weights pool
        psA = top.enter_context(tc.tile_pool(name="psA", bufs=8, space="PSUM"))

        def psum(shape, name):
            return psA.tile(shape, F32, tag="mm", name=name)

        wpool = top.enter_context(tc.tile_pool(name="wpool", bufs=2))

        main = top.enter_context(tc.tile_pool(name="main", bufs=1))
        o_fm = [main.tile([128, T], BF16, tag=f"o{i}", name=f"o{i}") for i in range(CT)]
        x2 = [main.tile([128, T], F32R, tag=f"x2_{i}", name=f"x2_{i}") for i in range(CT)]

        def ln_stats(xs, lp, jtag):
            """Column mean/rstd for one chunk -> broadcast (a_bc, b_bc); xh = x*a_bc - b_bc."""
            sq = []
            for i in range(CT):
                sqt = lp.tile([128, chunk], F32R, tag=f"sq{i & 1}", name=f"sq{i & 1}")
                nc.vector.tensor_mul(sqt[:], xs[i][:], xs[i][:])
                sq.append(sqt)
            s1 = psum([1, chunk], "s1")
            s2 = psum([1, chunk], "s2")
            for i in range(CT):
                nc.tensor.matmul(s1[:], ones_r[:], xs[i][:], start=(i == 0), stop=(i == CT - 1))
            for i in range(CT):
                nc.tensor.matmul(s2[:], ones_r[:], sq[i][:], start=(i == 0), stop=(i == CT - 1))
            mu = lp.tile([1, chunk], F32R, tag="mu", name="mu")
            nc.vector.tensor_scalar_mul(mu[:], s1[:], 1.0 / C)
            musq = lp.tile([1, chunk], F32R, tag="musq", name="musq")
            nc.vector.tensor_mul(musq[:], mu[:], mu[:])
            var = lp.tile([1, chunk], F32R, tag="var", name="var")
            nc.vector.tensor_scalar_mul(var[:], s2[:], 1.0 / C)
            nc.vector.tensor_sub(var[:], var[:], musq[:])
            sd = lp.tile([1, chunk], F32R, tag="sd", name="sd")
            nc.scalar.activation(out=sd[:], in_=var[:], func=AF.Sqrt, bias=eps_sb[:])
            rs = lp.tile([1, chunk], F32R, tag="rs", name="rs")
            with nc.allow_low_precision(reason="f32r is fp32-width"):
                nc.vector.reciprocal(rs[:], sd[:])
            murs = lp.tile([1, chunk], F32R, tag="murs", name="murs")
            nc.vector.tensor_mul(murs[:], mu[:], rs[:])
            a_bc = lp.tile([128, chunk], F32R, tag=f"a_bc{jtag}", name=f"a_bc{jtag}")
            nc.gpsimd.partition_broadcast(a_bc[:], rs[:])
            b_bc = lp.tile([128, chunk], F32R, tag=f"b_bc{jtag}", name=f"b_bc{jtag}")
            nc.gpsimd.partition_broadcast(b_bc[:], murs[:])
            return a_bc, b_bc

        def ln_apply(xs, a_bc, b_bc, dst, j):
            for i in range(CT):
                d = dst[i][:, j * chunk:(j + 1) * chunk]
                nc.vector.tensor_mul(d, xs[i][:], a_bc[:])
                nc.vector.tensor_sub(d, d, b_bc[:])

        def layernorm_into(src_tiles_of, dst, label, bufs=2):
            with tc.tile_pool(name=f"ln_{label}", bufs=bufs) as lp:
                for j in range(nch):
                    xs = src_tiles_of(j)
                    a_bc, b_bc = ln_stats(xs, lp, "")
                    ln_apply(xs, a_bc, b_bc, dst, j)

        lnstash = top.enter_context(tc.tile_pool(name="lnstash", bufs=1))

        ph1 = ExitStack()
        xhpool = ph1.enter_context(tc.tile_pool(name="xhpool", bufs=1))
        xh = [xhpool.tile([128, TP], F32R, tag=f"xh{i}", name=f"xh{i}") for i in range(CT)]

        # ---------------- LN1 -> xh ----------------
        with tc.tile_pool(name="xload", bufs=2) as xlp:
            def load_x_chunk(j):
                ts = []
                for i in range(CT):
                    t = xlp.tile([128, chunk], F32R, tag=f"x{i}", name=f"x{i}")
                    nc.sync.dma_start(t[:], x_fm[i * 128:(i + 1) * 128, j * chunk:(j + 1) * chunk])
                    ts.append(t)
                return ts
            layernorm_into(load_x_chunk, xh, "ln1", bufs=2)

        for i in range(CT):
            nc.vector.tensor_copy(xh[i][:, T:TP], zeros_f[:, 0:TP - T])

        # ---------------- q/k GEMM + V GEMM ----------------
        persist = ph1.enter_context(tc.tile_pool(name="qkvpool", bufs=1))
        q_sb = [persist.tile([128, TP], BF16, tag=f"q{i}", name=f"q{i}") for i in range(CT)]
        k_sb = [persist.tile([128, bl, KV], BF16, tag=f"k{i}", name=f"k{i}") for i in range(CT)]
        v_sb = [[persist.tile([128, H, Dh + 1], BF16, tag=f"v{im}_{pt}", name=f"v{im}_{pt}")
                 for pt in range(2)] for im in range(bl)]

        for mt in range(12):
            wt = wpool.tile([128, CT, 128], F32R, tag="w", name="w")
            nc.sync.dma_start(
                wt[:], w_qk.rearrange("(kt p) m -> p kt m", p=128)[:, :, mt * 128:(mt + 1) * 128])
            for j in range(nch):
                ps = psum([128, chunk], "ps")
                for i in range(CT):
                    nc.tensor.matmul(ps[:], wt[:, i, :], xh[i][:, j * chunk:(j + 1) * chunk],
                                     start=(i == 0), stop=(i == CT - 1))
                if mt < 6:
                    nc.vector.tensor_scalar_add(
                        q_sb[mt][:, j * chunk:(j + 1) * chunk], ps[:], bqk_sb[:, mt:mt + 1])
                else:
                    for v in range(chunk // N):
                        im = j * (chunk // N) + v
                        nc.vector.tensor_scalar_add(
                            k_sb[mt - 6][:, im, 0:N], ps[:, v * N:(v + 1) * N],
                            bqk_sb[:, mt:mt + 1])
        with tc.tile_pool(name="wvp", bufs=1) as wvp:
            for nt2 in range(2):
                wv_t = []
                for i in range(CT):
                    wti = wvp.tile([128, 384], F32R, tag=f"wv{i}", name=f"wv{i}")
                    nc.sync.dma_start(wti[:], w_v[i * 128:(i + 1) * 128, nt2 * 384:(nt2 + 1) * 384])
                    wv_t.append(wti)
                for im in range(bl):
                    for pt, (toff, tsz) in enumerate([(0, 128), (128, N - 128)]):
                        ps = psum([128, 384], "psv")
                        for i in range(CT):
                            nc.tensor.matmul(
                                ps[:tsz, :], xh[i][:, im * N + toff: im * N + toff + tsz],
                                wv_t[i][:], start=(i == 0), stop=(i == CT - 1))
                        nc.vector.tensor_copy(
                            v_sb[im][pt][:tsz, nt2 * 6:(nt2 + 1) * 6, 0:Dh],
                            ps[:tsz, :].rearrange("t (h d) -> t h d", d=Dh))
        for im in range(bl):
            nc.gpsimd.dma_start(v_sb[im][1][N - 128:N - 128 + P, :, 0:Dh], vp[im])
            for pt in range(2):
                nc.vector.tensor_copy(
                    v_sb[im][pt][:, :, Dh:Dh + 1],
                    ones_f[:, 0:1].to_broadcast([128, H, 1]))
        for i in range(CT):
            nc.vector.tensor_copy(q_sb[i][:, T:TP], zeros_f[:, 0:TP - T])
        for i in range(CT):
            nc.gpsimd.dma_start(k_sb[i][:, :, N:KV], kp[i])

        # ---------------- attention + proj interleaved per image pair ----------------
        wprp = ph1.enter_context(tc.tile_pool(name="wprp", bufs=1))
        wpr_t = []
        for mt in range(CT):
            wt = wprp.tile([128, CT, 128], BF16, tag=f"wpr{mt}", name=f"wpr{mt}")
            nc.sync.dma_start(
                wt[:], w_pr.rearrange("(kt p) m -> p kt m", p=128)[:, :, mt * 128:(mt + 1) * 128])
            wpr_t.append(wt)
        attn_ctx = ph1.enter_context(tc.tile_pool(name="attn", bufs=2))
        ln2ab = {}
        if True:
            ap = attn_ctx
            for im in range(bl):
                for h in range(H):
                    hp, hoff = h // 2, (h % 2) * 64
                    q_ap = q_sb[hp][hoff:hoff + 64, im * N: im * N + QP]
                    k_ap = k_sb[hp][hoff:hoff + 64, im, :]
                    st = psum([128, 2, QP], "st")
                    nc.tensor.matmul(st[:, 0, :], k_ap[:, 0:128], q_ap, start=True, stop=True)
                    nc.tensor.matmul(st[0:KV - 128, 1, :], k_ap[:, 128:KV], q_ap,
                                     start=True, stop=True)
                    e = ap.tile([128, 2, QP], BF16, tag="e", name="e")
                    nc.scalar.activation(out=e[:, 0, :], in_=st[:, 0, :], func=AF.Exp,
                                         scale=Dh ** -0.5)
                    nc.scalar.activation(out=e[0:KV - 128, 1, :], in_=st[0:KV - 128, 1, :],
                                         func=AF.Exp, scale=Dh ** -0.5)
                    av = psum([128, QP], "av")
                    nc.tensor.matmul(av[0:Dh + 1, :], v_sb[im][0][:, h, :], e[:, 0, :],
                                     start=True, stop=False)
                    nc.tensor.matmul(av[0:Dh + 1, :], v_sb[im][1][0:KV - 128, h, :],
                                     e[0:KV - 128, 1, :], start=False, stop=True)
                    rr = ap.tile([1, QP], F32R, tag="rr", name="rr")
                    with nc.allow_low_precision(reason="f32r is fp32-width"):
                        nc.vector.reciprocal(rr[:], av[Dh:Dh + 1, :])
                    rb = ap.tile([64, QP], F32R, tag="rb", name="rb")
                    nc.gpsimd.partition_broadcast(rb[:], rr[:])
                    o_ap = o_fm[hp][hoff:hoff + 64, im * N:(im + 1) * N]
                    nc.vector.tensor_mul(o_ap, av[0:Dh, 0:N], rb[:, 0:N])
                    nc.vector.tensor_scalar_add(o_ap, o_ap, bv_sb[hoff:hoff + 64, hp:hp + 1])
                if im % 2 == 1:
                    j = im // 2
                    for mt in range(CT):
                        ps = psum([128, chunk], "psp")
                        for i in range(CT):
                            nc.tensor.matmul(ps[:], wpr_t[mt][:, i, :],
                                             o_fm[i][:, j * chunk:(j + 1) * chunk],
                                             start=(i == 0), stop=(i == CT - 1))
                        xr = wpool.tile([128, chunk], F32, tag="w", name="xr")
                        nc.sync.dma_start(
                            xr[:], x_fm[mt * 128:(mt + 1) * 128, j * chunk:(j + 1) * chunk].bitcast(F32))
                        d = x2[mt][:, j * chunk:(j + 1) * chunk]
                        nc.vector.tensor_add(d, ps[:], xr[:])
                        nc.vector.tensor_scalar_add(d, d, bpr_sb[:, mt:mt + 1])
                    ln2ab[j] = ln_stats([x2[i][:, j * chunk:(j + 1) * chunk] for i in range(CT)],
                                        lnstash, f"_{j}")

        ph1.close()

        # ---------------- LN2 -> xh2, then MLP + residual -> out ----------------
        with ExitStack() as mlp:
            mpool = mlp.enter_context(tc.tile_pool(name="mlp", bufs=1))
            xh2 = [mpool.tile([128, T], F32R, tag=f"xh2_{i}", name=f"xh2_{i}") for i in range(CT)]
            for j in range(nch):
                a_bc, b_bc = ln2ab[j]
                ln_apply([x2[i][:, j * chunk:(j + 1) * chunk] for i in range(CT)],
                         a_bc, b_bc, xh2, j)

            gpool = mlp.enter_context(tc.tile_pool(name="gpool", bufs=1))
            g = [gpool.tile([128, T], BF16, tag=f"g{i}", name=f"g{i}") for i in range(Dff // 128)]

            for mt in range(Dff // 128):
                wt = wpool.tile([128, CT, 128], F32R, tag="w", name="w")
                nc.sync.dma_start(
                    wt[:], w_f1.rearrange("(kt p) m -> p kt m", p=128)[:, :, mt * 128:(mt + 1) * 128])
                for j in range(nch):
                    ps = psum([128, chunk], "ps1")
                    for i in range(CT):
                        nc.tensor.matmul(
                            ps[:], wt[:, i, :], xh2[i][:, j * chunk:(j + 1) * chunk],
                            start=(i == 0), stop=(i == CT - 1))
                    nc.scalar.activation(
                        out=g[mt][:, j * chunk:(j + 1) * chunk], in_=ps[:],
                        func=AF.Gelu, bias=bf1_sb[:, mt:mt + 1])
            with tc.tile_pool(name="ostage", bufs=2) as osp:
                for mt in range(CT):
                    wt = wpool.tile([128, Dff // 128, 128], BF16, tag="w", name="w")
                    nc.sync.dma_start(
                        wt[:], w_f2.rearrange("(kt p) m -> p kt m", p=128)[:, :, mt * 128:(mt + 1) * 128])
                    for j in range(nch):
                        ps = psum([128, chunk], "ps2")
                        for i in range(Dff // 128):
                            nc.tensor.matmul(ps[:], wt[:, i, :], g[i][:, j * chunk:(j + 1) * chunk],
                                             start=(i == 0), stop=(i == Dff // 128 - 1))
                        ot = osp.tile([128, chunk], F32, tag="ot", name="ot")
                        nc.vector.tensor_add(ot[:], ps[:], x2[mt][:, j * chunk:(j + 1) * chunk])
                        nc.vector.tensor_scalar_add(ot[:], ot[:], bf2_sb[:, mt:mt + 1])
                        nc.sync.dma_start(
                            out_fm[mt * 128:(mt + 1) * 128, j * chunk:(j + 1) * chunk], ot[:])

    nc.compile()
    return nc


_NC_CACHE = {}


def _get_nc(bl=BL):
    if bl not in _NC_CACHE:
        _NC_CACHE[bl] = build_nc(bl)
    return _NC_CACHE[bl]


def _host_prep(x, prompt, ln1_w, ln1_b, qkv_w, qkv_b, proj_w, proj_b,
               ln2_w, ln2_b, fc1_w, fc1_b, fc2_w, fc2_b, bl=BL, ncores=NCORES):
    import ml_dtypes
    f8 = np.float64
    ln1_w, ln1_b = f8(ln1_w), f8(ln1_b)
    ln2_w, ln2_b = f8(ln2_w), f8(ln2_b)
    qkv_w8, fc1_w8 = f8(qkv_w), f8(fc1_w)

    w_qk = np.ascontiguousarray((qkv_w8[:2 * C] * ln1_w).T.astype(np.float32))
    b_qk = (f8(qkv_b[:2 * C]) + qkv_w8[:2 * C] @ ln1_b).astype(np.float32).reshape(12, 128).T.copy()
    w_v = np.ascontiguousarray((qkv_w8[2 * C:] * ln1_w).T.astype(np.float32))
    b_v = (f8(qkv_b[2 * C:]) + qkv_w8[2 * C:] @ ln1_b).astype(np.float32).reshape(6, 128).T.copy()
    w_pr = np.ascontiguousarray(np.float32(proj_w).T.astype(ml_dtypes.bfloat16))
    b_pr = np.float32(proj_b).reshape(6, 128).T.copy()
    w_f1 = np.ascontiguousarray((fc1_w8 * ln2_w).T.astype(np.float32))
    b_f1 = (f8(fc1_b) + fc1_w8 @ ln2_b).astype(np.float32).reshape(24, 128).T.copy()
    w_f2 = np.ascontiguousarray(np.float32(fc2_w).T.astype(ml_dtypes.bfloat16))
    b_f2 = np.float32(fc2_b).reshape(6, 128).T.copy()

    shared = dict(w_qk=w_qk, b_qk=b_qk, w_v=w_v, b_v=b_v, w_pr=w_pr, b_pr=b_pr,
                  w_f1=w_f1, b_f1=b_f1, w_f2=w_f2, b_f2=b_f2)

    x = np.float32(x).reshape(ncores, bl, N, C)
    prompt = np.float32(prompt).reshape(ncores, bl, P, 2, H, Dh)
    in_maps = []
    for c in range(ncores):
        x_fm = np.ascontiguousarray(x[c].reshape(bl * N, C).T)
        kpc = prompt[c, :, :, 0]                         # [bl, P, H, Dh]
        kpa = np.ascontiguousarray(
            kpc.reshape(bl, P, CT, 128).transpose(2, 3, 0, 1)).astype(ml_dtypes.bfloat16)
        vpc = np.ascontiguousarray(prompt[c, :, :, 1]).astype(ml_dtypes.bfloat16)
        in_maps.append(dict(x_fm=x_fm, kp=kpa, vp=vpc, **shared))
    return in_maps


def run_sharded(inputs, bl=BL, ncores=NCORES, **spmd_kwargs):
    in_maps = _host_prep(**inputs, bl=bl, ncores=ncores)
    nc = _get_nc(bl)
    res = bass_utils.run_bass_kernel_spmd(nc, in_maps, core_ids=list(range(ncores)), **spmd_kwargs)
    outs = [r["out_fm"].T.reshape(bl, N, C) for r in res.results]
    return np.concatenate(outs, axis=0).astype(np.float32), res


def kernel(**inputs):
    out, _ = run_sharded(inputs, bl=BL, ncores=NCORES)
    return out
